# revision 80
# baseline (speedup 1.0000x reference)
"""Trainium2 Bass kernel for FASTMultiHeadAttention (fastmax, Taylor-2 softmax approx
with relative positional embeddings, optional causal mask).

B=1, H=8, N=2048, D=64. One head per NeuronCore (8 cores).

Math per head (q,k,v: [N,D], rpe: [2N-1, D]):
    s[i,j]  = q_i.k_j + q_i.rpe[i-j+N-1]
    w       = 1 + s + s^2/2      (causal-masked if mask)
    out_i   = sum_j w[i,j] v_j / sum_j w[i,j]

Device algorithm (per head):
    w = ((s+1)^2 + 1)/2 on valid entries, so with t = (s+1)^2 (t=0 on masked):
      numer_i = 0.5*(sum_j t_ij v_j + vcum_i)
      denom_i = 0.5*(sum_j t_ij + (i+1))
    The +1 inside the square comes from a 65th "ones" contraction row: qT/kT
    carry a ones row (rpe a zeros row), so the content matmul yields q.k + 1
    and the full score s1 = (q.k + 1) + q.rpe needs only a tensor_tensor add.

    - content+rpe scores: K=65 f32r matmuls (fp32 PSUM)
    - rpe diagonal realignment R[a,j] = QR[a, 127-a+j]: skewed SBUF->SBUF DMA
    - s1 = S_psum + R  via tensor_tensor on DVE (single PSUM input)
    - causal mask: affine_select zeroes j > i on the diagonal chunk (Pool)
    - W^T via PE transposes (bf16), squared during PSUM evacuation
      (ACT Square; hardware allows only one PSUM input per instruction)
    - O = sum_j t v via PE matmul with V (+ones col) stationary, K=128
    - normalize with host-precomputed vcum/iota, store [128, NT*64] row-major
"""

import sys
import os
import numpy as np

for _p in ("/opt/trn_rl_repo", "/root/.axon_site/_ro/trn_rl_repo"):
    if os.path.isdir(_p) and _p not in sys.path:
        sys.path.insert(0, _p)

B, H, N, D = 1, 8, 2048, 64
NT = N // 128            # 16 i-tiles of 128 rows
NJC = N // 128           # 16 j-chunks of 128 (for transposes / O matmul)

_CACHE = {}

# engine-assignment tuning (fractions routed to the listed engine)
TUNE = {
    "sq_act_frac": 1.0,     # (unused; squares are ACT-only, PSUM 1-input rule)
    "qr_dve_frac": 0.42,     # QR psum->sbuf copies on DVE (else ACT)
    "qr_pool_frac": 0.0,    # unused: GPSIMD cannot access PSUM
    "s1_pool_frac": 0.0,    # s1 TT chunks on Pool (else DVE)
    "gather_act_frac": 0.0, # gathers issued from ACT ring (else SP)
    "norm_pool": True,      # normalize adds on Pool (else DVE)
    "ot_dve_frac": 0.0,     # oT psum->sbuf evacs on DVE (else ACT)
    "swap_tail": False,      # process tile 7 last (short fin chain)
    "qrp_bufs": 4,
    "rrp_bufs": 6,
    "s1p_bufs": 4,
}


class _Frac:
    # weighted deterministic router: pick() True with rate `frac`
    def __init__(self, frac):
        self.f = frac
        self.acc = 0.0

    def pick(self):
        self.acc += self.f
        if self.acc >= 0.999:
            self.acc -= 1.0
            return True
        return False


def _build_program(causal: bool, reps: int = 1):
    import concourse.bass as bass
    from concourse import bacc
    import concourse.mybir as mybir
    from concourse.tile import TileContext
    from concourse.masks import make_identity

    fp32 = mybir.dt.float32
    f32r = mybir.dt.float32r
    bf16 = mybir.dt.bfloat16
    AT = mybir.ActivationFunctionType
    OP = mybir.AluOpType

    RPW = 2560 if causal else 4608   # rpe_revT padded width

    nc = bacc.Bacc("TRN2", target_bir_lowering=False, debug=False)

    qT_d = nc.dram_tensor("qT", [65, N], f32r, kind="ExternalInput")
    kT_d = nc.dram_tensor("kT", [65, N], f32r, kind="ExternalInput")
    v_d = nc.dram_tensor("vr", [128, NJC * 66], bf16, kind="ExternalInput")
    vcum_d = nc.dram_tensor("vcum", [128, NT * 64], fp32, kind="ExternalInput")
    rpe_d = nc.dram_tensor("rpeT", [65, RPW], f32r, kind="ExternalInput")
    iota_d = nc.dram_tensor("iota", [128, NT], fp32, kind="ExternalInput")
    o_d = nc.dram_tensor("o", [128, NT * 64], fp32, kind="ExternalOutput")

    def j_max(t):
        return 128 * (t + 1) if causal else N

    def u_min(t):
        return (N - 1) - 128 * t - 127

    def qr_w(t):
        return 127 + j_max(t)

    with TileContext(nc) as tc:
        with (
            tc.tile_pool(name="persist", bufs=1) as pp,
            tc.tile_pool(name="qr", bufs=TUNE["qrp_bufs"]) as qrp,
            tc.tile_pool(name="rr", bufs=TUNE["rrp_bufs"]) as rrp,
            tc.tile_pool(name="s1", bufs=TUNE["s1p_bufs"]) as s1p,
            tc.tile_pool(name="small", bufs=2) as sp,
        ):
            sq_r = _Frac(TUNE["sq_act_frac"])
            qrd_r = _Frac(TUNE["qr_dve_frac"])
            qrp_r = _Frac(TUNE["qr_pool_frac"])
            s1p_r = _Frac(TUNE["s1_pool_frac"])
            ga_r = _Frac(TUNE["gather_act_frac"])
            ot_r = _Frac(TUNE["ot_dve_frac"])

            # ---- persistent tiles ----
            qT_s = pp.tile([65, N], f32r, name="qT_s")
            kT_s = pp.tile([65, N], f32r, name="kT_s")
            rpe_s = pp.tile([65, RPW], f32r, name="rpe_s")
            v_s = pp.tile([128, NJC * 66], bf16, name="v_s")
            vcum_s = pp.tile([128, NT * 64], fp32, name="vcum_s")
            iota_s = pp.tile([128, NT], fp32, name="iota_s")

            # chunked loads, ordered by pipeline consumption under the
            # interleaved tile order (small tile t, then tile t+8, ...)
            if causal:
                rpe_chunks = ((1920, 2176), (896, 1920), (0, 896), (2176, 2304))
            else:
                rpe_chunks = ((896, RPW), (0, 896))
            qT_chunks = ((0, 128), (1024, 1152), (128, 1024), (1152, 2048))
            kT_chunks = ((0, 256), (256, 1280), (1280, 2048))
            # fill-critical chunks on SP first (tiles 0 and 8 consume them
            # within the first two iterations); the rest on Pool SWDGE / ACT
            nc.sync.dma_start(out=qT_s[:, 0:128], in_=qT_d.ap()[:, 0:128])
            nc.scalar.dma_start(out=rpe_s[:, rpe_chunks[0][0]:rpe_chunks[0][1]],
                                in_=rpe_d.ap()[:, rpe_chunks[0][0]:rpe_chunks[0][1]])
            nc.sync.dma_start(out=qT_s[:, 1024:1152], in_=qT_d.ap()[:, 1024:1152])
            nc.sync.dma_start(out=kT_s[:, 0:256], in_=kT_d.ap()[:, 0:256])
            nc.sync.dma_start(out=rpe_s[:, rpe_chunks[1][0]:rpe_chunks[1][1]],
                              in_=rpe_d.ap()[:, rpe_chunks[1][0]:rpe_chunks[1][1]])
            def bulk_loads_a():
                # consumed first: qT for tiles 1/9, kT body
                for c0, c1 in qT_chunks[2:]:
                    nc.sync.dma_start(out=qT_s[:, c0:c1], in_=qT_d.ap()[:, c0:c1])
                for c0, c1 in kT_chunks[1:]:
                    nc.sync.dma_start(out=kT_s[:, c0:c1], in_=kT_d.ap()[:, c0:c1])
                nc.gpsimd.dma_start(out=v_s[:], in_=v_d.ap())

            def bulk_loads_b():
                for c0, c1 in rpe_chunks[2:]:
                    nc.sync.dma_start(out=rpe_s[:, c0:c1], in_=rpe_d.ap()[:, c0:c1])
                nc.gpsimd.dma_start(out=vcum_s[:], in_=vcum_d.ap())
                nc.gpsimd.dma_start(out=iota_s[:], in_=iota_d.ap())

            bulk_loads_a()
            bulk_loads_b()

            ident = pp.tile([128, 128], bf16, name="ident")
            make_identity(nc, ident[:])
            ident66_f = pp.tile([66, 66], fp32, name="ident66_f")
            make_identity(nc, ident66_f[:])
            ident66_r = pp.tile([66, 66], f32r, name="ident66_r")
            nc.vector.tensor_copy(ident66_r[:], ident66_f[:])
            ident66 = ident66_r[:]

            # W^T storage, triangular-packed by groups of 4 j-chunks when causal:
            # group g0 stores only i >= 128*g0 (width Wg = N - 128*g0).
            def wt_imin(jc):
                return 128 * (4 * (jc // 4)) if causal else 0

            def wt_w(jc):
                return N - wt_imin(jc)

            _wt_base = {}
            _off = 0
            for _jc in range(NJC):
                _wt_base[_jc] = _off
                _off += wt_w(_jc)
            WTW = _off
            wt_all = pp.tile([128, WTW], bf16, name="wt_all")

            out_s = pp.tile([128, NT * 64], fp32, name="out_s")

            for _rep in range(reps):
              with (
                  tc.tile_pool(name="qr_ps", bufs=2, space="PSUM") as qrps,
                  tc.tile_pool(name="s_ps", bufs=2, space="PSUM") as sps,
                  tc.tile_pool(name="tr_ps", bufs=2, space="PSUM") as trp,
              ):
                live = {}

                def mm65(out_ps, t, src, c0, mw):
                    i0 = 128 * t
                    nc.tensor.matmul(out_ps, qT_s[:, i0:i0 + 128],
                                     src[:, c0:c0 + mw],
                                     start=True, stop=True, tile_position=(0, 0))

                def stageA(t):
                    # rpe projection QR (K=65, zero row kills the ones term),
                    # fp32 psum, ACT evac to bf16, then diagonal gather of R
                    w = qr_w(t)
                    um = u_min(t)
                    qrbuf = qrp.tile([128, 2560 if causal else 2304], bf16, name="qrbuf")
                    for b0 in range(0, w, 1024):
                        bw = min(1024, w - b0)
                        qr_ps = qrps.tile([128, 1024], fp32, name="qr_ps")
                        for h0 in range(0, bw, 512):
                            hw = min(512, bw - h0)
                            mw = max(256, (hw + 1) & ~1)  # f32r ISA: even, >= 256
                            mm65(qr_ps[:, h0:h0 + mw], t, rpe_s, um + b0 + h0, mw)
                        if qrd_r.pick():
                            nc.vector.tensor_copy(qrbuf[:, b0:b0 + bw], qr_ps[:, 0:bw])
                        elif qrp_r.pick():
                            nc.gpsimd.tensor_copy(qrbuf[:, b0:b0 + bw], qr_ps[:, 0:bw])
                        else:
                            nc.scalar.activation(qrbuf[:, b0:b0 + bw], qr_ps[:, 0:bw],
                                                 AT.Copy, bias=0.0, scale=1.0)
                    # diagonal gather R[a, j] = qrbuf[a, 127 - a + j]
                    QW = qrbuf[:].tensor.shape[1]
                    R_row = rrp.tile([128, N], bf16, name="R_row")
                    diag = bass.AP(qrbuf[:].tensor, qrbuf[:].offset + 127,
                                   [[QW - 1, 128], [1, j_max(t)]])
                    eng = nc.scalar if ga_r.pick() else nc.sync
                    eng.dma_start(out=R_row[:, 0:j_max(t)], in_=diag)
                    live[("A", t)] = R_row

                def stageB(t):
                    # content scores (K=65 with ones row -> q.k + 1), bf16 psum,
                    # s1 = S + R via DVE tensor_tensor (2x), causal mask on diag
                    i0 = 128 * t
                    jm = j_max(t)
                    R_row = live.pop(("A", t))
                    s1_row = s1p.tile([128, N], bf16, name="s1_row", tag="s1_row")
                    for jb in range(0, jm, 512):
                        cw = min(512, jm - jb)
                        s_ps = sps.tile([128, 512], fp32, name="s_ps")
                        mw = max(256, (cw + 1) & ~1)
                        mm65(s_ps[:, 0:mw], t, kT_s, jb, mw)
                        teng = nc.gpsimd if s1p_r.pick() else nc.vector
                        teng.tensor_tensor(
                            out=s1_row[:, jb:jb + cw], in0=s_ps[:, 0:cw],
                            in1=R_row[:, jb:jb + cw], op=OP.add)
                    s1_diag = None
                    if causal:
                        # masked diagonal chunk goes to its own tile so the mask
                        # doesn't gate the other chunks' transposes
                        s1_diag = s1p.tile([128, 128], bf16, name="s1_diag", tag="s1_diag")
                        nc.gpsimd.affine_select(
                            out=s1_diag[:], in_=s1_row[:, i0:i0 + 128],
                            compare_op=OP.is_ge, fill=0.0,
                            base=0, channel_multiplier=1, pattern=[[-1, 128]])
                    live[("B", t)] = (s1_diag, s1_row)

                def stageC(t):
                    # transpose s1 chunks, square during PSUM evacuation -> wt_all
                    i0 = 128 * t
                    s1_diag, s1_row = live.pop(("B", t))
                    njc = (j_max(t) + 127) // 128
                    for g0 in range(0, njc, 4):
                        gn = min(4, njc - g0)
                        tr_ps = trp.tile([128, 512], bf16, name="tr_ps")
                        for g in range(gn):
                            jc = g0 + g
                            src_chunk = (s1_diag[:] if (causal and jc == t)
                                         else s1_row[:, 128 * jc:128 * (jc + 1)])
                            nc.tensor.transpose(tr_ps[:, 128 * g:128 * (g + 1)],
                                                src_chunk, ident[:])
                        dst = bass.AP(wt_all[:].tensor,
                                      wt_all[:].offset + _wt_base[g0] + (i0 - wt_imin(g0)),
                                      [[WTW, 128], [wt_w(g0), gn], [1, 128]])
                        srcap = tr_ps[:, 0:128 * gn].rearrange("p (g c) -> p g c", g=gn)
                        # PSUM allows only one tensor input per instruction, so
                        # the square must be ACT's single-input Square
                        nc.scalar.activation(dst, srcap, AT.Square, bias=0.0, scale=1.0)

                def stageOacc(t):
                    # accumulate O for i-range [128t, 128t+128) over its j-chunks
                    # right after stageC(t) wrote those W^T columns; transient
                    # psum partial, evacuated straight to the slab SBUF tile
                    s = t // 4
                    if ("O", s) not in live:
                        live[("O", s)] = sp.tile([66, 512], f32r, name="oT_s", tag="oT_s")
                    oT_s = live[("O", s)]
                    c0 = 128 * (t % 4)
                    o_ps = sps.tile([66, 128], fp32, name="s_ps")
                    jc_hi = t + 1 if causal else NJC
                    for jc in range(jc_hi):
                        rhs = bass.AP(wt_all[:].tensor,
                                      wt_all[:].offset + _wt_base[jc] + (128 * t - wt_imin(jc)),
                                      [[WTW, 128], [1, 128]])
                        nc.tensor.matmul(o_ps[:, 0:128],
                                         v_s[:, 66 * jc:66 * (jc + 1)], rhs,
                                         start=(jc == 0), stop=(jc == jc_hi - 1))
                    if ot_r.pick():
                        nc.vector.tensor_copy(oT_s[:, c0:c0 + 128], o_ps[:, 0:128])
                    else:
                        nc.scalar.activation(oT_s[:, c0:c0 + 128], o_ps[:, 0:128],
                                             AT.Copy, bias=0.0, scale=1.0)

                def stageOfin(s):
                    # back-transpose + normalize + store slab s
                    t0, t1 = 4 * s, 4 * s + 4
                    oT_s = live.pop(("O", s))
                    ob_ps = qrps.tile([128, 264], f32r, name="qr_ps")
                    for g in range(4):
                        nc.tensor.transpose(ob_ps[:, 66 * g:66 * (g + 1)],
                                            oT_s[:, 128 * g:128 * (g + 1)],
                                            ident66)
                    # normalize straight from the back-transpose psum (one PSUM
                    # input per instruction is legal on DVE)
                    obf = ob_ps[:].bitcast(fp32)
                    dtot = sp.tile([128, 4], fp32, name="dtot", tag="dtot")
                    dcol = bass.AP(obf.tensor, obf.offset + 64, [[264, 128], [66, 4]])
                    nc.vector.tensor_tensor(out=dtot[:], in0=dcol, in1=iota_s[:, t0:t1], op=OP.add)
                    recip = sp.tile([128, 4], fp32, name="recip", tag="recip")
                    nc.vector.reciprocal(recip[:], dtot[:])
                    onum = bass.AP(obf.tensor, obf.offset, [[264, 128], [66, 4], [1, 64]])
                    osl = out_s[:, 64 * t0:64 * t1].rearrange("p (t d) -> p t d", d=64)
                    nc.vector.tensor_tensor(
                        out=osl, in0=onum,
                        in1=vcum_s[:, 64 * t0:64 * t1].rearrange("p (t d) -> p t d", d=64),
                        op=OP.add)
                    rb = bass.AP(recip[:].tensor, recip[:].offset, [[4, 128], [1, 4], [0, 64]])
                    neng = nc.gpsimd if TUNE["norm_pool"] else nc.vector
                    neng.tensor_tensor(out=osl, in0=osl, in1=rb, op=OP.mult)
                    nc.sync.dma_start(out=o_d.ap()[:, 64 * t0:64 * t1],
                                      in_=out_s[:, 64 * t0:64 * t1])

                # interleaved tile order pairs small and large tiles so the
                # per-iteration engine load is roughly uniform
                order = [t for pair in zip(range(NT // 2), range(NT // 2, NT))
                         for t in pair]
                if TUNE["swap_tail"]:
                    order[-2], order[-1] = order[-1], order[-2]
                slab_done = {s: 0 for s in range(NT // 4)}
                for u in range(NT + 4):
                    if 2 <= u < NT + 2:
                        stageB(order[u - 2])
                    if u < NT:
                        stageA(order[u])
                    if u >= 4:
                        t = order[u - 4]
                        stageC(t)
                        stageOacc(t)
                        slab_done[t // 4] += 1
                        if slab_done[t // 4] == 4:
                            stageOfin(t // 4)

    nc.compile()
    return nc


def _make_runner(nc, n_cores):
    import concourse.mybir as mybir
    import jax
    from jax.sharding import Mesh, PartitionSpec
    from jax.experimental.shard_map import shard_map
    from concourse.bass2jax import install_neuronx_cc_hook, _bass_exec_p, partition_id_tensor

    install_neuronx_cc_hook()
    partition_name = nc.partition_id_tensor.name if nc.partition_id_tensor else None
    in_names, out_names, out_avals, zero_outs = [], [], [], []
    for alloc in nc.m.functions[0].allocations:
        if not isinstance(alloc, mybir.MemoryLocationSet):
            continue
        name = alloc.memorylocations[0].name
        if alloc.kind == "ExternalInput":
            if name != partition_name:
                in_names.append(name)
        elif alloc.kind == "ExternalOutput":
            shape = tuple(alloc.tensor_shape)
            dtype = mybir.dt.np(alloc.dtype)
            out_names.append(name)
            out_avals.append(jax.core.ShapedArray(shape, dtype))
            zero_outs.append(np.zeros(shape, dtype))
    n_params = len(in_names)
    n_outs = len(out_avals)
    all_in_names = list(in_names) + list(out_names)
    if partition_name is not None:
        all_in_names.append(partition_name)

    def _body(*args):
        operands = list(args)
        if partition_name is not None:
            operands.append(partition_id_tensor())
        outs = _bass_exec_p.bind(
            *operands, out_avals=tuple(out_avals), in_names=tuple(all_in_names),
            out_names=tuple(out_names), lowering_input_output_aliases=(),
            sim_require_finite=True, sim_require_nnan=True, nc=nc)
        return tuple(outs)

    devices = jax.devices()[:n_cores]
    mesh = Mesh(np.asarray(devices), ("core",))
    in_specs = (PartitionSpec("core"),) * (n_params + n_outs)
    out_specs = (PartitionSpec("core"),) * n_outs
    jitted = jax.jit(shard_map(_body, mesh=mesh, in_specs=in_specs,
                               out_specs=out_specs, check_rep=False), keep_unused=True)

    def run(in_maps):
        concat_in = [np.concatenate([np.asarray(in_maps[c][n]) for c in range(n_cores)], axis=0)
                     for n in in_names]
        concat_zeros = [np.zeros((n_cores * z.shape[0], *z.shape[1:]), z.dtype) for z in zero_outs]
        outs = jitted(*concat_in, *concat_zeros)
        import jax as _jax
        _jax.block_until_ready(outs)
        return [{name: np.asarray(outs[i]).reshape(n_cores, *out_avals[i].shape)[c]
                 for i, name in enumerate(out_names)} for c in range(n_cores)]
    return run


def _get_runner(causal: bool):
    key = bool(causal)
    if key not in _CACHE:
        nc = _build_program(key)
        _CACHE[key] = _make_runner(nc, H)
    return _CACHE[key]


def _prep_head(q2, k2, v2, causal):
    """q2,k2,v2: [N, D] fp32 for one head. Returns per-core input dict."""
    import ml_dtypes
    qT = np.concatenate([q2.T, np.ones((1, N), np.float32)], axis=0)  # [65, N]
    kT = np.concatenate([k2.T, np.ones((1, N), np.float32)], axis=0)
    # v with ones col 64 (denominator) and zero col 65 (f32r even-width pad)
    v3 = np.concatenate([v2, np.ones((N, 1), np.float32),
                         np.zeros((N, 1), np.float32)], axis=1)      # [N, 66]
    v_r = np.ascontiguousarray(
        v3.reshape(NJC, 128, 66).transpose(1, 0, 2).reshape(128, NJC * 66)
    ).astype(ml_dtypes.bfloat16)
    if causal:
        vc = np.cumsum(v2, axis=0, dtype=np.float64).astype(np.float32)
    else:
        vc = np.broadcast_to(v2.sum(axis=0, dtype=np.float64).astype(np.float32), (N, 64))
    vcum = np.ascontiguousarray(
        vc.reshape(NT, 128, 64).transpose(1, 0, 2).reshape(128, NT * 64))
    return {"qT": np.ascontiguousarray(qT), "kT": np.ascontiguousarray(kT),
            "vr": v_r, "vcum": vcum}


def kernel(q, k, v, rpe_matrix, mask):
    causal = bool(np.asarray(mask).item()) if not isinstance(mask, (int, bool)) else bool(mask)
    q = np.asarray(q, dtype=np.float32)
    k = np.asarray(k, dtype=np.float32)
    v = np.asarray(v, dtype=np.float32)
    rpe = np.asarray(rpe_matrix, dtype=np.float32)

    RPW = 2560 if causal else 4608
    if causal:
        # u in [0, N-1]: rpe_rev[u] = rpe[2N-2-u] -> rows 2N-2 .. N-1
        rpe_rev = rpe[N - 1:2 * N - 1][::-1]             # [N, 64]
    else:
        rpe_rev = rpe[::-1]                              # [2N-1, 64]
    rpeT = np.zeros((65, RPW), dtype=np.float32)
    rpeT[0:64, :rpe_rev.shape[0]] = rpe_rev.T            # row 64 stays zero

    a = np.arange(128, dtype=np.float32)[:, None]
    tt = np.arange(NT, dtype=np.float32)[None, :]
    iota = (128 * tt + a + 1.0) if causal else np.full((128, NT), float(N), np.float32)
    iota = np.ascontiguousarray(iota.astype(np.float32))

    run = _get_runner(causal)
    in_maps = []
    for h in range(H):
        m = _prep_head(q[0, h], k[0, h], v[0, h], causal)
        m["rpeT"] = rpeT
        m["iota"] = iota
        in_maps.append(m)
    results = run(in_maps)
    # o stored [128, NT*64] with o_store[a, 64t+d] = o[128t+a, d]
    outs = []
    for h in range(H):
        oh = results[h]["o"].reshape(128, NT, 64).transpose(1, 0, 2).reshape(N, 64)
        outs.append(oh)
    out = np.stack(outs)[None]  # [1, H, N, 64]
    return out.astype(np.float32)


if __name__ == "__main__":
    rng = np.random.default_rng(0)
    q = rng.standard_normal((B, H, N, D), dtype=np.float32)
    k = rng.standard_normal((B, H, N, D), dtype=np.float32)
    v = rng.standard_normal((B, H, N, D), dtype=np.float32)
    rpe = rng.standard_normal((2 * N - 1, D), dtype=np.float32)
    o = kernel(q, k, v, rpe, 1)
    print("out", o.shape, o.dtype, np.abs(o).mean())


# revision 81
# speedup vs baseline: 1.0009x; 1.0009x over previous
"""Trainium2 Bass kernel for FASTMultiHeadAttention (fastmax, Taylor-2 softmax approx
with relative positional embeddings, optional causal mask).

B=1, H=8, N=2048, D=64. One head per NeuronCore (8 cores).

Math per head (q,k,v: [N,D], rpe: [2N-1, D]):
    s[i,j]  = q_i.k_j + q_i.rpe[i-j+N-1]
    w       = 1 + s + s^2/2      (causal-masked if mask)
    out_i   = sum_j w[i,j] v_j / sum_j w[i,j]

Device algorithm (per head):
    w = ((s+1)^2 + 1)/2 on valid entries, so with t = (s+1)^2 (t=0 on masked):
      numer_i = 0.5*(sum_j t_ij v_j + vcum_i)
      denom_i = 0.5*(sum_j t_ij + (i+1))
    The +1 inside the square comes from a 65th "ones" contraction row: qT/kT
    carry a ones row (rpe a zeros row), so the content matmul yields q.k + 1
    and the full score s1 = (q.k + 1) + q.rpe needs only a tensor_tensor add.

    - content+rpe scores: K=65 f32r matmuls (fp32 PSUM)
    - rpe diagonal realignment R[a,j] = QR[a, 127-a+j]: skewed SBUF->SBUF DMA
    - s1 = S_psum + R  via tensor_tensor on DVE (single PSUM input)
    - causal mask: affine_select zeroes j > i on the diagonal chunk (Pool)
    - W^T via PE transposes (bf16), squared during PSUM evacuation
      (ACT Square; hardware allows only one PSUM input per instruction)
    - O = sum_j t v via PE matmul with V (+ones col) stationary, K=128
    - normalize with host-precomputed vcum/iota, store [128, NT*64] row-major
"""

import sys
import os
import numpy as np

for _p in ("/opt/trn_rl_repo", "/root/.axon_site/_ro/trn_rl_repo"):
    if os.path.isdir(_p) and _p not in sys.path:
        sys.path.insert(0, _p)

B, H, N, D = 1, 8, 2048, 64
NT = N // 128            # 16 i-tiles of 128 rows
NJC = N // 128           # 16 j-chunks of 128 (for transposes / O matmul)

_CACHE = {}

# engine-assignment tuning (fractions routed to the listed engine)
TUNE = {
    "sq_act_frac": 1.0,     # (unused; squares are ACT-only, PSUM 1-input rule)
    "qr_dve_frac": 0.42,     # QR psum->sbuf copies on DVE (else ACT)
    "qr_pool_frac": 0.0,    # unused: GPSIMD cannot access PSUM
    "s1_pool_frac": 0.0,    # s1 TT chunks on Pool (else DVE)
    "gather_act_frac": 0.0, # gathers issued from ACT ring (else SP)
    "norm_pool": True,      # normalize adds on Pool (else DVE)
    "ot_dve_frac": 0.0,     # oT psum->sbuf evacs on DVE (else ACT)
    "swap_tail": False,      # process tile 7 last (short fin chain)
    "qrp_bufs": 4,
    "rrp_bufs": 6,
    "s1p_bufs": 4,
}


class _Frac:
    # weighted deterministic router: pick() True with rate `frac`
    def __init__(self, frac):
        self.f = frac
        self.acc = 0.0

    def pick(self):
        self.acc += self.f
        if self.acc >= 0.999:
            self.acc -= 1.0
            return True
        return False


def _build_program(causal: bool, reps: int = 1):
    import concourse.bass as bass
    from concourse import bacc
    import concourse.mybir as mybir
    from concourse.tile import TileContext
    from concourse.masks import make_identity

    fp32 = mybir.dt.float32
    f32r = mybir.dt.float32r
    bf16 = mybir.dt.bfloat16
    AT = mybir.ActivationFunctionType
    OP = mybir.AluOpType

    RPW = 2560 if causal else 4608   # rpe_revT padded width

    nc = bacc.Bacc("TRN2", target_bir_lowering=False, debug=False)

    qT_d = nc.dram_tensor("qT", [65, N], f32r, kind="ExternalInput")
    kT_d = nc.dram_tensor("kT", [65, N], f32r, kind="ExternalInput")
    v_d = nc.dram_tensor("vr", [128, NJC * 66], bf16, kind="ExternalInput")
    vcum_d = nc.dram_tensor("vcum", [128, NT * 64], fp32, kind="ExternalInput")
    rpe_d = nc.dram_tensor("rpeT", [65, RPW], f32r, kind="ExternalInput")
    iota_d = nc.dram_tensor("iota", [128, NT], fp32, kind="ExternalInput")
    o_d = nc.dram_tensor("o", [128, NT * 64], fp32, kind="ExternalOutput")

    def j_max(t):
        return 128 * (t + 1) if causal else N

    def u_min(t):
        return (N - 1) - 128 * t - 127

    def qr_w(t):
        return 127 + j_max(t)

    with TileContext(nc) as tc:
        with (
            tc.tile_pool(name="persist", bufs=1) as pp,
            tc.tile_pool(name="qr", bufs=TUNE["qrp_bufs"]) as qrp,
            tc.tile_pool(name="rr", bufs=TUNE["rrp_bufs"]) as rrp,
            tc.tile_pool(name="s1", bufs=TUNE["s1p_bufs"]) as s1p,
            tc.tile_pool(name="small", bufs=2) as sp,
        ):
            sq_r = _Frac(TUNE["sq_act_frac"])
            qrd_r = _Frac(TUNE["qr_dve_frac"])
            qrp_r = _Frac(TUNE["qr_pool_frac"])
            s1p_r = _Frac(TUNE["s1_pool_frac"])
            ga_r = _Frac(TUNE["gather_act_frac"])
            ot_r = _Frac(TUNE["ot_dve_frac"])

            # ---- persistent tiles ----
            qT_s = pp.tile([65, N], f32r, name="qT_s")
            kT_s = pp.tile([65, N], f32r, name="kT_s")
            rpe_s = pp.tile([65, RPW], f32r, name="rpe_s")
            v_s = pp.tile([128, NJC * 66], bf16, name="v_s")
            vcum_s = pp.tile([128, NT * 64], fp32, name="vcum_s")
            iota_s = pp.tile([128, NT], fp32, name="iota_s")

            # chunked loads, ordered by pipeline consumption under the
            # interleaved tile order (small tile t, then tile t+8, ...)
            if causal:
                rpe_chunks = ((1920, 2176), (896, 1920), (0, 896), (2176, 2304))
            else:
                rpe_chunks = ((896, RPW), (0, 896))
            qT_chunks = ((0, 128), (1024, 1152), (128, 1024), (1152, 2048))
            kT_chunks = ((0, 256), (256, 1280), (1280, 2048))
            # fill-critical chunks on SP first (tiles 0 and 8 consume them
            # within the first two iterations); the rest on Pool SWDGE / ACT
            nc.sync.dma_start(out=qT_s[:, 0:128], in_=qT_d.ap()[:, 0:128])
            nc.scalar.dma_start(out=rpe_s[:, rpe_chunks[0][0]:rpe_chunks[0][1]],
                                in_=rpe_d.ap()[:, rpe_chunks[0][0]:rpe_chunks[0][1]])
            nc.sync.dma_start(out=qT_s[:, 1024:1152], in_=qT_d.ap()[:, 1024:1152])
            nc.sync.dma_start(out=kT_s[:, 0:256], in_=kT_d.ap()[:, 0:256])
            nc.sync.dma_start(out=rpe_s[:, rpe_chunks[1][0]:rpe_chunks[1][1]],
                              in_=rpe_d.ap()[:, rpe_chunks[1][0]:rpe_chunks[1][1]])
            def bulk_loads_a():
                # consumed first: qT for tiles 1/9, kT body
                for c0, c1 in qT_chunks[2:]:
                    nc.sync.dma_start(out=qT_s[:, c0:c1], in_=qT_d.ap()[:, c0:c1])
                for c0, c1 in kT_chunks[1:]:
                    nc.sync.dma_start(out=kT_s[:, c0:c1], in_=kT_d.ap()[:, c0:c1])
                nc.gpsimd.dma_start(out=v_s[:], in_=v_d.ap())

            def bulk_loads_b():
                for c0, c1 in rpe_chunks[2:]:
                    nc.sync.dma_start(out=rpe_s[:, c0:c1], in_=rpe_d.ap()[:, c0:c1])
                nc.gpsimd.dma_start(out=vcum_s[:], in_=vcum_d.ap())
                nc.gpsimd.dma_start(out=iota_s[:], in_=iota_d.ap())

            bulk_loads_a()
            bulk_loads_b()

            ident = pp.tile([128, 128], bf16, name="ident")
            make_identity(nc, ident[:])
            ident66_f = pp.tile([66, 66], fp32, name="ident66_f")
            make_identity(nc, ident66_f[:])
            ident66_r = pp.tile([66, 66], f32r, name="ident66_r")
            nc.vector.tensor_copy(ident66_r[:], ident66_f[:])
            ident66 = ident66_r[:]

            # W^T storage, triangular-packed by groups of 4 j-chunks when causal:
            # group g0 stores only i >= 128*g0 (width Wg = N - 128*g0).
            def wt_imin(jc):
                return 128 * (4 * (jc // 4)) if causal else 0

            def wt_w(jc):
                return N - wt_imin(jc)

            _wt_base = {}
            _off = 0
            for _jc in range(NJC):
                _wt_base[_jc] = _off
                _off += wt_w(_jc)
            WTW = _off
            wt_all = pp.tile([128, WTW], bf16, name="wt_all")

            out_s = pp.tile([128, NT * 64], fp32, name="out_s")

            for _rep in range(reps):
              with (
                  tc.tile_pool(name="qr_ps", bufs=2, space="PSUM") as qrps,
                  tc.tile_pool(name="s_ps", bufs=2, space="PSUM") as sps,
                  tc.tile_pool(name="tr_ps", bufs=2, space="PSUM") as trp,
              ):
                live = {}

                def mm65(out_ps, t, src, c0, mw):
                    i0 = 128 * t
                    nc.tensor.matmul(out_ps, qT_s[:, i0:i0 + 128],
                                     src[:, c0:c0 + mw],
                                     start=True, stop=True, tile_position=(0, 0))

                def stageA(t):
                    # rpe projection QR (K=65, zero row kills the ones term),
                    # fp32 psum, ACT evac to bf16, then diagonal gather of R
                    w = qr_w(t)
                    um = u_min(t)
                    qrbuf = qrp.tile([128, 2560 if causal else 2304], bf16, name="qrbuf")
                    for b0 in range(0, w, 1024):
                        bw = min(1024, w - b0)
                        qr_ps = qrps.tile([128, 1024], fp32, name="qr_ps")
                        for h0 in range(0, bw, 512):
                            hw = min(512, bw - h0)
                            mw = max(256, (hw + 1) & ~1)  # f32r ISA: even, >= 256
                            mm65(qr_ps[:, h0:h0 + mw], t, rpe_s, um + b0 + h0, mw)
                        if qrd_r.pick():
                            nc.vector.tensor_copy(qrbuf[:, b0:b0 + bw], qr_ps[:, 0:bw])
                        elif qrp_r.pick():
                            nc.gpsimd.tensor_copy(qrbuf[:, b0:b0 + bw], qr_ps[:, 0:bw])
                        else:
                            nc.scalar.activation(qrbuf[:, b0:b0 + bw], qr_ps[:, 0:bw],
                                                 AT.Copy, bias=0.0, scale=1.0)
                    # diagonal gather R[a, j] = qrbuf[a, 127 - a + j]; split in
                    # two so low-j columns ship after the first evac chunk
                    QW = qrbuf[:].tensor.shape[1]
                    R_row = rrp.tile([128, N], bf16, name="R_row")
                    jm = j_max(t)
                    jsplit = min(1024 - 127, jm) if w > 1024 else jm
                    for ja, jb_ in ((0, jsplit), (jsplit, jm)):
                        if jb_ <= ja:
                            continue
                        diag = bass.AP(qrbuf[:].tensor, qrbuf[:].offset + 127 + ja,
                                       [[QW - 1, 128], [1, jb_ - ja]])
                        eng = nc.scalar if ga_r.pick() else nc.sync
                        eng.dma_start(out=R_row[:, ja:jb_], in_=diag)
                    live[("A", t)] = R_row

                def stageB(t):
                    # content scores (K=65 with ones row -> q.k + 1), bf16 psum,
                    # s1 = S + R via DVE tensor_tensor (2x), causal mask on diag
                    i0 = 128 * t
                    jm = j_max(t)
                    R_row = live.pop(("A", t))
                    s1_row = s1p.tile([128, N], bf16, name="s1_row", tag="s1_row")
                    for jb in range(0, jm, 512):
                        cw = min(512, jm - jb)
                        s_ps = sps.tile([128, 512], fp32, name="s_ps")
                        mw = max(256, (cw + 1) & ~1)
                        mm65(s_ps[:, 0:mw], t, kT_s, jb, mw)
                        teng = nc.gpsimd if s1p_r.pick() else nc.vector
                        teng.tensor_tensor(
                            out=s1_row[:, jb:jb + cw], in0=s_ps[:, 0:cw],
                            in1=R_row[:, jb:jb + cw], op=OP.add)
                    s1_diag = None
                    if causal:
                        # masked diagonal chunk goes to its own tile so the mask
                        # doesn't gate the other chunks' transposes
                        s1_diag = s1p.tile([128, 128], bf16, name="s1_diag", tag="s1_diag")
                        nc.gpsimd.affine_select(
                            out=s1_diag[:], in_=s1_row[:, i0:i0 + 128],
                            compare_op=OP.is_ge, fill=0.0,
                            base=0, channel_multiplier=1, pattern=[[-1, 128]])
                    live[("B", t)] = (s1_diag, s1_row)

                def stageC(t):
                    # transpose s1 chunks, square during PSUM evacuation -> wt_all
                    i0 = 128 * t
                    s1_diag, s1_row = live.pop(("B", t))
                    njc = (j_max(t) + 127) // 128
                    for g0 in range(0, njc, 4):
                        gn = min(4, njc - g0)
                        tr_ps = trp.tile([128, 512], bf16, name="tr_ps")
                        for g in range(gn):
                            jc = g0 + g
                            src_chunk = (s1_diag[:] if (causal and jc == t)
                                         else s1_row[:, 128 * jc:128 * (jc + 1)])
                            nc.tensor.transpose(tr_ps[:, 128 * g:128 * (g + 1)],
                                                src_chunk, ident[:])
                        dst = bass.AP(wt_all[:].tensor,
                                      wt_all[:].offset + _wt_base[g0] + (i0 - wt_imin(g0)),
                                      [[WTW, 128], [wt_w(g0), gn], [1, 128]])
                        srcap = tr_ps[:, 0:128 * gn].rearrange("p (g c) -> p g c", g=gn)
                        # PSUM allows only one tensor input per instruction, so
                        # the square must be ACT's single-input Square
                        nc.scalar.activation(dst, srcap, AT.Square, bias=0.0, scale=1.0)

                def stageOacc(t):
                    # accumulate O for i-range [128t, 128t+128) over its j-chunks
                    # right after stageC(t) wrote those W^T columns; transient
                    # psum partial, evacuated straight to the slab SBUF tile
                    s = t // 4
                    if ("O", s) not in live:
                        live[("O", s)] = sp.tile([66, 512], f32r, name="oT_s", tag="oT_s")
                    oT_s = live[("O", s)]
                    c0 = 128 * (t % 4)
                    o_ps = sps.tile([66, 128], fp32, name="s_ps")
                    jc_hi = t + 1 if causal else NJC
                    for jc in range(jc_hi):
                        rhs = bass.AP(wt_all[:].tensor,
                                      wt_all[:].offset + _wt_base[jc] + (128 * t - wt_imin(jc)),
                                      [[WTW, 128], [1, 128]])
                        nc.tensor.matmul(o_ps[:, 0:128],
                                         v_s[:, 66 * jc:66 * (jc + 1)], rhs,
                                         start=(jc == 0), stop=(jc == jc_hi - 1))
                    if ot_r.pick():
                        nc.vector.tensor_copy(oT_s[:, c0:c0 + 128], o_ps[:, 0:128])
                    else:
                        nc.scalar.activation(oT_s[:, c0:c0 + 128], o_ps[:, 0:128],
                                             AT.Copy, bias=0.0, scale=1.0)

                def stageOfin(s):
                    # back-transpose + normalize + store slab s
                    t0, t1 = 4 * s, 4 * s + 4
                    oT_s = live.pop(("O", s))
                    ob_ps = qrps.tile([128, 264], f32r, name="qr_ps")
                    for g in range(4):
                        nc.tensor.transpose(ob_ps[:, 66 * g:66 * (g + 1)],
                                            oT_s[:, 128 * g:128 * (g + 1)],
                                            ident66)
                    # normalize straight from the back-transpose psum (one PSUM
                    # input per instruction is legal on DVE)
                    obf = ob_ps[:].bitcast(fp32)
                    dtot = sp.tile([128, 4], fp32, name="dtot", tag="dtot")
                    dcol = bass.AP(obf.tensor, obf.offset + 64, [[264, 128], [66, 4]])
                    nc.vector.tensor_tensor(out=dtot[:], in0=dcol, in1=iota_s[:, t0:t1], op=OP.add)
                    recip = sp.tile([128, 4], fp32, name="recip", tag="recip")
                    nc.vector.reciprocal(recip[:], dtot[:])
                    onum = bass.AP(obf.tensor, obf.offset, [[264, 128], [66, 4], [1, 64]])
                    osl = out_s[:, 64 * t0:64 * t1].rearrange("p (t d) -> p t d", d=64)
                    nc.vector.tensor_tensor(
                        out=osl, in0=onum,
                        in1=vcum_s[:, 64 * t0:64 * t1].rearrange("p (t d) -> p t d", d=64),
                        op=OP.add)
                    rb = bass.AP(recip[:].tensor, recip[:].offset, [[4, 128], [1, 4], [0, 64]])
                    neng = nc.gpsimd if TUNE["norm_pool"] else nc.vector
                    neng.tensor_tensor(out=osl, in0=osl, in1=rb, op=OP.mult)
                    nc.sync.dma_start(out=o_d.ap()[:, 64 * t0:64 * t1],
                                      in_=out_s[:, 64 * t0:64 * t1])

                # interleaved tile order pairs small and large tiles so the
                # per-iteration engine load is roughly uniform
                order = [t for pair in zip(range(NT // 2), range(NT // 2, NT))
                         for t in pair]
                if TUNE["swap_tail"]:
                    order[-2], order[-1] = order[-1], order[-2]
                slab_done = {s: 0 for s in range(NT // 4)}
                for u in range(NT + 4):
                    if 2 <= u < NT + 2:
                        stageB(order[u - 2])
                    if u < NT:
                        stageA(order[u])
                    if u >= 4:
                        t = order[u - 4]
                        stageC(t)
                        stageOacc(t)
                        slab_done[t // 4] += 1
                        if slab_done[t // 4] == 4:
                            stageOfin(t // 4)

    nc.compile()
    return nc


def _make_runner(nc, n_cores):
    import concourse.mybir as mybir
    import jax
    from jax.sharding import Mesh, PartitionSpec
    from jax.experimental.shard_map import shard_map
    from concourse.bass2jax import install_neuronx_cc_hook, _bass_exec_p, partition_id_tensor

    install_neuronx_cc_hook()
    partition_name = nc.partition_id_tensor.name if nc.partition_id_tensor else None
    in_names, out_names, out_avals, zero_outs = [], [], [], []
    for alloc in nc.m.functions[0].allocations:
        if not isinstance(alloc, mybir.MemoryLocationSet):
            continue
        name = alloc.memorylocations[0].name
        if alloc.kind == "ExternalInput":
            if name != partition_name:
                in_names.append(name)
        elif alloc.kind == "ExternalOutput":
            shape = tuple(alloc.tensor_shape)
            dtype = mybir.dt.np(alloc.dtype)
            out_names.append(name)
            out_avals.append(jax.core.ShapedArray(shape, dtype))
            zero_outs.append(np.zeros(shape, dtype))
    n_params = len(in_names)
    n_outs = len(out_avals)
    all_in_names = list(in_names) + list(out_names)
    if partition_name is not None:
        all_in_names.append(partition_name)

    def _body(*args):
        operands = list(args)
        if partition_name is not None:
            operands.append(partition_id_tensor())
        outs = _bass_exec_p.bind(
            *operands, out_avals=tuple(out_avals), in_names=tuple(all_in_names),
            out_names=tuple(out_names), lowering_input_output_aliases=(),
            sim_require_finite=True, sim_require_nnan=True, nc=nc)
        return tuple(outs)

    devices = jax.devices()[:n_cores]
    mesh = Mesh(np.asarray(devices), ("core",))
    in_specs = (PartitionSpec("core"),) * (n_params + n_outs)
    out_specs = (PartitionSpec("core"),) * n_outs
    jitted = jax.jit(shard_map(_body, mesh=mesh, in_specs=in_specs,
                               out_specs=out_specs, check_rep=False), keep_unused=True)

    def run(in_maps):
        concat_in = [np.concatenate([np.asarray(in_maps[c][n]) for c in range(n_cores)], axis=0)
                     for n in in_names]
        concat_zeros = [np.zeros((n_cores * z.shape[0], *z.shape[1:]), z.dtype) for z in zero_outs]
        outs = jitted(*concat_in, *concat_zeros)
        import jax as _jax
        _jax.block_until_ready(outs)
        return [{name: np.asarray(outs[i]).reshape(n_cores, *out_avals[i].shape)[c]
                 for i, name in enumerate(out_names)} for c in range(n_cores)]
    return run


def _get_runner(causal: bool):
    key = bool(causal)
    if key not in _CACHE:
        nc = _build_program(key)
        _CACHE[key] = _make_runner(nc, H)
    return _CACHE[key]


def _prep_head(q2, k2, v2, causal):
    """q2,k2,v2: [N, D] fp32 for one head. Returns per-core input dict."""
    import ml_dtypes
    qT = np.concatenate([q2.T, np.ones((1, N), np.float32)], axis=0)  # [65, N]
    kT = np.concatenate([k2.T, np.ones((1, N), np.float32)], axis=0)
    # v with ones col 64 (denominator) and zero col 65 (f32r even-width pad)
    v3 = np.concatenate([v2, np.ones((N, 1), np.float32),
                         np.zeros((N, 1), np.float32)], axis=1)      # [N, 66]
    v_r = np.ascontiguousarray(
        v3.reshape(NJC, 128, 66).transpose(1, 0, 2).reshape(128, NJC * 66)
    ).astype(ml_dtypes.bfloat16)
    if causal:
        vc = np.cumsum(v2, axis=0, dtype=np.float64).astype(np.float32)
    else:
        vc = np.broadcast_to(v2.sum(axis=0, dtype=np.float64).astype(np.float32), (N, 64))
    vcum = np.ascontiguousarray(
        vc.reshape(NT, 128, 64).transpose(1, 0, 2).reshape(128, NT * 64))
    return {"qT": np.ascontiguousarray(qT), "kT": np.ascontiguousarray(kT),
            "vr": v_r, "vcum": vcum}


def kernel(q, k, v, rpe_matrix, mask):
    causal = bool(np.asarray(mask).item()) if not isinstance(mask, (int, bool)) else bool(mask)
    q = np.asarray(q, dtype=np.float32)
    k = np.asarray(k, dtype=np.float32)
    v = np.asarray(v, dtype=np.float32)
    rpe = np.asarray(rpe_matrix, dtype=np.float32)

    RPW = 2560 if causal else 4608
    if causal:
        # u in [0, N-1]: rpe_rev[u] = rpe[2N-2-u] -> rows 2N-2 .. N-1
        rpe_rev = rpe[N - 1:2 * N - 1][::-1]             # [N, 64]
    else:
        rpe_rev = rpe[::-1]                              # [2N-1, 64]
    rpeT = np.zeros((65, RPW), dtype=np.float32)
    rpeT[0:64, :rpe_rev.shape[0]] = rpe_rev.T            # row 64 stays zero

    a = np.arange(128, dtype=np.float32)[:, None]
    tt = np.arange(NT, dtype=np.float32)[None, :]
    iota = (128 * tt + a + 1.0) if causal else np.full((128, NT), float(N), np.float32)
    iota = np.ascontiguousarray(iota.astype(np.float32))

    run = _get_runner(causal)
    in_maps = []
    for h in range(H):
        m = _prep_head(q[0, h], k[0, h], v[0, h], causal)
        m["rpeT"] = rpeT
        m["iota"] = iota
        in_maps.append(m)
    results = run(in_maps)
    # o stored [128, NT*64] with o_store[a, 64t+d] = o[128t+a, d]
    outs = []
    for h in range(H):
        oh = results[h]["o"].reshape(128, NT, 64).transpose(1, 0, 2).reshape(N, 64)
        outs.append(oh)
    out = np.stack(outs)[None]  # [1, H, N, 64]
    return out.astype(np.float32)


if __name__ == "__main__":
    rng = np.random.default_rng(0)
    q = rng.standard_normal((B, H, N, D), dtype=np.float32)
    k = rng.standard_normal((B, H, N, D), dtype=np.float32)
    v = rng.standard_normal((B, H, N, D), dtype=np.float32)
    rpe = rng.standard_normal((2 * N - 1, D), dtype=np.float32)
    o = kernel(q, k, v, rpe, 1)
    print("out", o.shape, o.dtype, np.abs(o).mean())


# revision 82
# speedup vs baseline: 1.0055x; 1.0046x over previous
"""Trainium2 Bass kernel for FASTMultiHeadAttention (fastmax, Taylor-2 softmax approx
with relative positional embeddings, optional causal mask).

B=1, H=8, N=2048, D=64. One head per NeuronCore (8 cores).

Math per head (q,k,v: [N,D], rpe: [2N-1, D]):
    s[i,j]  = q_i.k_j + q_i.rpe[i-j+N-1]
    w       = 1 + s + s^2/2      (causal-masked if mask)
    out_i   = sum_j w[i,j] v_j / sum_j w[i,j]

Device algorithm (per head):
    w = ((s+1)^2 + 1)/2 on valid entries, so with t = (s+1)^2 (t=0 on masked):
      numer_i = 0.5*(sum_j t_ij v_j + vcum_i)
      denom_i = 0.5*(sum_j t_ij + (i+1))
    The +1 inside the square comes from a 65th "ones" contraction row: qT/kT
    carry a ones row (rpe a zeros row), so the content matmul yields q.k + 1
    and the full score s1 = (q.k + 1) + q.rpe needs only a tensor_tensor add.

    - content+rpe scores: K=65 f32r matmuls (fp32 PSUM)
    - rpe diagonal realignment R[a,j] = QR[a, 127-a+j]: skewed SBUF->SBUF DMA
    - s1 = S_psum + R  via tensor_tensor on DVE (single PSUM input)
    - causal mask: affine_select zeroes j > i on the diagonal chunk (Pool)
    - W^T via PE transposes (bf16), squared during PSUM evacuation
      (ACT Square; hardware allows only one PSUM input per instruction)
    - O = sum_j t v via PE matmul with V (+ones col) stationary, K=128
    - normalize with host-precomputed vcum/iota, store [128, NT*64] row-major
"""

import sys
import os
import numpy as np

for _p in ("/opt/trn_rl_repo", "/root/.axon_site/_ro/trn_rl_repo"):
    if os.path.isdir(_p) and _p not in sys.path:
        sys.path.insert(0, _p)

B, H, N, D = 1, 8, 2048, 64
NT = N // 128            # 16 i-tiles of 128 rows
NJC = N // 128           # 16 j-chunks of 128 (for transposes / O matmul)

_CACHE = {}

# engine-assignment tuning (fractions routed to the listed engine)
TUNE = {
    "sq_act_frac": 1.0,     # (unused; squares are ACT-only, PSUM 1-input rule)
    "qr_dve_frac": 0.42,     # QR psum->sbuf copies on DVE (else ACT)
    "qr_pool_frac": 0.0,    # unused: GPSIMD cannot access PSUM
    "s1_pool_frac": 0.0,    # s1 TT chunks on Pool (else DVE)
    "gather_act_frac": 0.0, # gathers issued from ACT ring (else SP)
    "norm_pool": True,      # normalize adds on Pool (else DVE)
    "ot_dve_frac": 0.25,     # oT psum->sbuf evacs on DVE (else ACT)
    "swap_tail": False,      # process tile 7 last (short fin chain)
    "qrp_bufs": 4,
    "rrp_bufs": 6,
    "s1p_bufs": 4,
}


class _Frac:
    # weighted deterministic router: pick() True with rate `frac`
    def __init__(self, frac):
        self.f = frac
        self.acc = 0.0

    def pick(self):
        self.acc += self.f
        if self.acc >= 0.999:
            self.acc -= 1.0
            return True
        return False


def _build_program(causal: bool, reps: int = 1):
    import concourse.bass as bass
    from concourse import bacc
    import concourse.mybir as mybir
    from concourse.tile import TileContext
    from concourse.masks import make_identity

    fp32 = mybir.dt.float32
    f32r = mybir.dt.float32r
    bf16 = mybir.dt.bfloat16
    AT = mybir.ActivationFunctionType
    OP = mybir.AluOpType

    RPW = 2560 if causal else 4608   # rpe_revT padded width

    nc = bacc.Bacc("TRN2", target_bir_lowering=False, debug=False)

    qT_d = nc.dram_tensor("qT", [65, N], f32r, kind="ExternalInput")
    kT_d = nc.dram_tensor("kT", [65, N], f32r, kind="ExternalInput")
    v_d = nc.dram_tensor("vr", [128, NJC * 66], bf16, kind="ExternalInput")
    vcum_d = nc.dram_tensor("vcum", [128, NT * 64], fp32, kind="ExternalInput")
    rpe_d = nc.dram_tensor("rpeT", [65, RPW], f32r, kind="ExternalInput")
    iota_d = nc.dram_tensor("iota", [128, NT], fp32, kind="ExternalInput")
    o_d = nc.dram_tensor("o", [128, NT * 64], fp32, kind="ExternalOutput")

    def j_max(t):
        return 128 * (t + 1) if causal else N

    def u_min(t):
        return (N - 1) - 128 * t - 127

    def qr_w(t):
        return 127 + j_max(t)

    with TileContext(nc) as tc:
        with (
            tc.tile_pool(name="persist", bufs=1) as pp,
            tc.tile_pool(name="qr", bufs=TUNE["qrp_bufs"]) as qrp,
            tc.tile_pool(name="rr", bufs=TUNE["rrp_bufs"]) as rrp,
            tc.tile_pool(name="s1", bufs=TUNE["s1p_bufs"]) as s1p,
            tc.tile_pool(name="small", bufs=2) as sp,
        ):
            sq_r = _Frac(TUNE["sq_act_frac"])
            qrd_r = _Frac(TUNE["qr_dve_frac"])
            qrp_r = _Frac(TUNE["qr_pool_frac"])
            s1p_r = _Frac(TUNE["s1_pool_frac"])
            ga_r = _Frac(TUNE["gather_act_frac"])
            ot_r = _Frac(TUNE["ot_dve_frac"])

            # ---- persistent tiles ----
            qT_s = pp.tile([65, N], f32r, name="qT_s")
            kT_s = pp.tile([65, N], f32r, name="kT_s")
            rpe_s = pp.tile([65, RPW], f32r, name="rpe_s")
            v_s = pp.tile([128, NJC * 66], bf16, name="v_s")
            vcum_s = pp.tile([128, NT * 64], fp32, name="vcum_s")
            iota_s = pp.tile([128, NT], fp32, name="iota_s")

            # chunked loads, ordered by pipeline consumption under the
            # interleaved tile order (small tile t, then tile t+8, ...)
            if causal:
                rpe_chunks = ((1920, 2176), (896, 1920), (0, 896), (2176, 2304))
            else:
                rpe_chunks = ((896, RPW), (0, 896))
            qT_chunks = ((0, 128), (1024, 1152), (128, 1024), (1152, 2048))
            kT_chunks = ((0, 256), (256, 1280), (1280, 2048))
            # fill-critical chunks on SP first (tiles 0 and 8 consume them
            # within the first two iterations); the rest on Pool SWDGE / ACT
            nc.sync.dma_start(out=qT_s[:, 0:128], in_=qT_d.ap()[:, 0:128])
            nc.scalar.dma_start(out=rpe_s[:, rpe_chunks[0][0]:rpe_chunks[0][1]],
                                in_=rpe_d.ap()[:, rpe_chunks[0][0]:rpe_chunks[0][1]])
            nc.sync.dma_start(out=qT_s[:, 1024:1152], in_=qT_d.ap()[:, 1024:1152])
            nc.sync.dma_start(out=kT_s[:, 0:256], in_=kT_d.ap()[:, 0:256])
            nc.sync.dma_start(out=rpe_s[:, rpe_chunks[1][0]:rpe_chunks[1][1]],
                              in_=rpe_d.ap()[:, rpe_chunks[1][0]:rpe_chunks[1][1]])
            def bulk_loads_a():
                # consumed first: qT for tiles 1/9, kT body
                for c0, c1 in qT_chunks[2:]:
                    nc.sync.dma_start(out=qT_s[:, c0:c1], in_=qT_d.ap()[:, c0:c1])
                for c0, c1 in kT_chunks[1:]:
                    nc.sync.dma_start(out=kT_s[:, c0:c1], in_=kT_d.ap()[:, c0:c1])
                nc.gpsimd.dma_start(out=v_s[:], in_=v_d.ap())

            def bulk_loads_b():
                for c0, c1 in rpe_chunks[2:]:
                    nc.sync.dma_start(out=rpe_s[:, c0:c1], in_=rpe_d.ap()[:, c0:c1])
                nc.gpsimd.dma_start(out=vcum_s[:], in_=vcum_d.ap())
                nc.gpsimd.dma_start(out=iota_s[:], in_=iota_d.ap())

            bulk_loads_a()
            bulk_loads_b()

            ident = pp.tile([128, 128], bf16, name="ident")
            make_identity(nc, ident[:])
            ident66_f = pp.tile([66, 66], fp32, name="ident66_f")
            make_identity(nc, ident66_f[:])
            ident66_r = pp.tile([66, 66], f32r, name="ident66_r")
            nc.vector.tensor_copy(ident66_r[:], ident66_f[:])
            ident66 = ident66_r[:]

            # W^T storage, triangular-packed by groups of 4 j-chunks when causal:
            # group g0 stores only i >= 128*g0 (width Wg = N - 128*g0).
            def wt_imin(jc):
                return 128 * (4 * (jc // 4)) if causal else 0

            def wt_w(jc):
                return N - wt_imin(jc)

            _wt_base = {}
            _off = 0
            for _jc in range(NJC):
                _wt_base[_jc] = _off
                _off += wt_w(_jc)
            WTW = _off
            wt_all = pp.tile([128, WTW], bf16, name="wt_all")

            out_s = pp.tile([128, NT * 64], fp32, name="out_s")

            for _rep in range(reps):
              with (
                  tc.tile_pool(name="qr_ps", bufs=2, space="PSUM") as qrps,
                  tc.tile_pool(name="s_ps", bufs=2, space="PSUM") as sps,
                  tc.tile_pool(name="tr_ps", bufs=2, space="PSUM") as trp,
              ):
                live = {}

                def mm65(out_ps, t, src, c0, mw):
                    i0 = 128 * t
                    nc.tensor.matmul(out_ps, qT_s[:, i0:i0 + 128],
                                     src[:, c0:c0 + mw],
                                     start=True, stop=True, tile_position=(0, 0))

                def stageA(t):
                    # rpe projection QR (K=65, zero row kills the ones term),
                    # fp32 psum, ACT evac to bf16, then diagonal gather of R
                    w = qr_w(t)
                    um = u_min(t)
                    qrbuf = qrp.tile([128, 2560 if causal else 2304], bf16, name="qrbuf")
                    for b0 in range(0, w, 1024):
                        bw = min(1024, w - b0)
                        qr_ps = qrps.tile([128, 1024], fp32, name="qr_ps")
                        for h0 in range(0, bw, 512):
                            hw = min(512, bw - h0)
                            mw = max(256, (hw + 1) & ~1)  # f32r ISA: even, >= 256
                            mm65(qr_ps[:, h0:h0 + mw], t, rpe_s, um + b0 + h0, mw)
                        if qrd_r.pick():
                            nc.vector.tensor_copy(qrbuf[:, b0:b0 + bw], qr_ps[:, 0:bw])
                        elif qrp_r.pick():
                            nc.gpsimd.tensor_copy(qrbuf[:, b0:b0 + bw], qr_ps[:, 0:bw])
                        else:
                            nc.scalar.activation(qrbuf[:, b0:b0 + bw], qr_ps[:, 0:bw],
                                                 AT.Copy, bias=0.0, scale=1.0)
                    # diagonal gather R[a, j] = qrbuf[a, 127 - a + j]; split in
                    # two so low-j columns ship after the first evac chunk
                    QW = qrbuf[:].tensor.shape[1]
                    R_row = rrp.tile([128, N], bf16, name="R_row")
                    jm = j_max(t)
                    jsplit = min(1024 - 127, jm) if w > 1024 else jm
                    for ja, jb_ in ((0, jsplit), (jsplit, jm)):
                        if jb_ <= ja:
                            continue
                        diag = bass.AP(qrbuf[:].tensor, qrbuf[:].offset + 127 + ja,
                                       [[QW - 1, 128], [1, jb_ - ja]])
                        eng = nc.scalar if ga_r.pick() else nc.sync
                        eng.dma_start(out=R_row[:, ja:jb_], in_=diag)
                    live[("A", t)] = R_row

                def stageB(t):
                    # content scores (K=65 with ones row -> q.k + 1), bf16 psum,
                    # s1 = S + R via DVE tensor_tensor (2x), causal mask on diag
                    i0 = 128 * t
                    jm = j_max(t)
                    R_row = live.pop(("A", t))
                    s1_row = s1p.tile([128, N], bf16, name="s1_row", tag="s1_row")
                    for jb in range(0, jm, 512):
                        cw = min(512, jm - jb)
                        s_ps = sps.tile([128, 512], fp32, name="s_ps")
                        mw = max(256, (cw + 1) & ~1)
                        mm65(s_ps[:, 0:mw], t, kT_s, jb, mw)
                        teng = nc.gpsimd if s1p_r.pick() else nc.vector
                        teng.tensor_tensor(
                            out=s1_row[:, jb:jb + cw], in0=s_ps[:, 0:cw],
                            in1=R_row[:, jb:jb + cw], op=OP.add)
                    s1_diag = None
                    if causal:
                        # masked diagonal chunk goes to its own tile so the mask
                        # doesn't gate the other chunks' transposes
                        s1_diag = s1p.tile([128, 128], bf16, name="s1_diag", tag="s1_diag")
                        nc.gpsimd.affine_select(
                            out=s1_diag[:], in_=s1_row[:, i0:i0 + 128],
                            compare_op=OP.is_ge, fill=0.0,
                            base=0, channel_multiplier=1, pattern=[[-1, 128]])
                    live[("B", t)] = (s1_diag, s1_row)

                def stageC(t):
                    # transpose s1 chunks, square during PSUM evacuation -> wt_all
                    i0 = 128 * t
                    s1_diag, s1_row = live.pop(("B", t))
                    njc = (j_max(t) + 127) // 128
                    for g0 in range(0, njc, 4):
                        gn = min(4, njc - g0)
                        tr_ps = trp.tile([128, 512], bf16, name="tr_ps")
                        for g in range(gn):
                            jc = g0 + g
                            src_chunk = (s1_diag[:] if (causal and jc == t)
                                         else s1_row[:, 128 * jc:128 * (jc + 1)])
                            nc.tensor.transpose(tr_ps[:, 128 * g:128 * (g + 1)],
                                                src_chunk, ident[:])
                        dst = bass.AP(wt_all[:].tensor,
                                      wt_all[:].offset + _wt_base[g0] + (i0 - wt_imin(g0)),
                                      [[WTW, 128], [wt_w(g0), gn], [1, 128]])
                        srcap = tr_ps[:, 0:128 * gn].rearrange("p (g c) -> p g c", g=gn)
                        # PSUM allows only one tensor input per instruction, so
                        # the square must be ACT's single-input Square
                        nc.scalar.activation(dst, srcap, AT.Square, bias=0.0, scale=1.0)

                def stageOacc(t):
                    # accumulate O for i-range [128t, 128t+128) over its j-chunks
                    # right after stageC(t) wrote those W^T columns; transient
                    # psum partial, evacuated straight to the slab SBUF tile
                    s = t // 4
                    if ("O", s) not in live:
                        live[("O", s)] = sp.tile([66, 512], f32r, name="oT_s", tag="oT_s")
                    oT_s = live[("O", s)]
                    c0 = 128 * (t % 4)
                    o_ps = sps.tile([66, 128], fp32, name="s_ps")
                    jc_hi = t + 1 if causal else NJC
                    for jc in range(jc_hi):
                        rhs = bass.AP(wt_all[:].tensor,
                                      wt_all[:].offset + _wt_base[jc] + (128 * t - wt_imin(jc)),
                                      [[WTW, 128], [1, 128]])
                        nc.tensor.matmul(o_ps[:, 0:128],
                                         v_s[:, 66 * jc:66 * (jc + 1)], rhs,
                                         start=(jc == 0), stop=(jc == jc_hi - 1))
                    if ot_r.pick():
                        nc.vector.tensor_copy(oT_s[:, c0:c0 + 128], o_ps[:, 0:128])
                    else:
                        nc.scalar.activation(oT_s[:, c0:c0 + 128], o_ps[:, 0:128],
                                             AT.Copy, bias=0.0, scale=1.0)

                def stageOfin(s):
                    # back-transpose + normalize + store slab s
                    t0, t1 = 4 * s, 4 * s + 4
                    oT_s = live.pop(("O", s))
                    ob_ps = qrps.tile([128, 264], f32r, name="qr_ps")
                    for g in range(4):
                        nc.tensor.transpose(ob_ps[:, 66 * g:66 * (g + 1)],
                                            oT_s[:, 128 * g:128 * (g + 1)],
                                            ident66)
                    # normalize straight from the back-transpose psum (one PSUM
                    # input per instruction is legal on DVE)
                    obf = ob_ps[:].bitcast(fp32)
                    dtot = sp.tile([128, 4], fp32, name="dtot", tag="dtot")
                    dcol = bass.AP(obf.tensor, obf.offset + 64, [[264, 128], [66, 4]])
                    nc.vector.tensor_tensor(out=dtot[:], in0=dcol, in1=iota_s[:, t0:t1], op=OP.add)
                    recip = sp.tile([128, 4], fp32, name="recip", tag="recip")
                    nc.vector.reciprocal(recip[:], dtot[:])
                    onum = bass.AP(obf.tensor, obf.offset, [[264, 128], [66, 4], [1, 64]])
                    osl = out_s[:, 64 * t0:64 * t1].rearrange("p (t d) -> p t d", d=64)
                    nc.vector.tensor_tensor(
                        out=osl, in0=onum,
                        in1=vcum_s[:, 64 * t0:64 * t1].rearrange("p (t d) -> p t d", d=64),
                        op=OP.add)
                    rb = bass.AP(recip[:].tensor, recip[:].offset, [[4, 128], [1, 4], [0, 64]])
                    neng = nc.gpsimd if TUNE["norm_pool"] else nc.vector
                    neng.tensor_tensor(out=osl, in0=osl, in1=rb, op=OP.mult)
                    nc.sync.dma_start(out=o_d.ap()[:, 64 * t0:64 * t1],
                                      in_=out_s[:, 64 * t0:64 * t1])

                # interleaved tile order pairs small and large tiles so the
                # per-iteration engine load is roughly uniform
                order = [t for pair in zip(range(NT // 2), range(NT // 2, NT))
                         for t in pair]
                if TUNE["swap_tail"]:
                    order[-2], order[-1] = order[-1], order[-2]
                slab_done = {s: 0 for s in range(NT // 4)}
                for u in range(NT + 4):
                    if 2 <= u < NT + 2:
                        stageB(order[u - 2])
                    if u < NT:
                        stageA(order[u])
                    if u >= 4:
                        t = order[u - 4]
                        stageC(t)
                        stageOacc(t)
                        slab_done[t // 4] += 1
                        if slab_done[t // 4] == 4:
                            stageOfin(t // 4)

    nc.compile()
    return nc


def _make_runner(nc, n_cores):
    import concourse.mybir as mybir
    import jax
    from jax.sharding import Mesh, PartitionSpec
    from jax.experimental.shard_map import shard_map
    from concourse.bass2jax import install_neuronx_cc_hook, _bass_exec_p, partition_id_tensor

    install_neuronx_cc_hook()
    partition_name = nc.partition_id_tensor.name if nc.partition_id_tensor else None
    in_names, out_names, out_avals, zero_outs = [], [], [], []
    for alloc in nc.m.functions[0].allocations:
        if not isinstance(alloc, mybir.MemoryLocationSet):
            continue
        name = alloc.memorylocations[0].name
        if alloc.kind == "ExternalInput":
            if name != partition_name:
                in_names.append(name)
        elif alloc.kind == "ExternalOutput":
            shape = tuple(alloc.tensor_shape)
            dtype = mybir.dt.np(alloc.dtype)
            out_names.append(name)
            out_avals.append(jax.core.ShapedArray(shape, dtype))
            zero_outs.append(np.zeros(shape, dtype))
    n_params = len(in_names)
    n_outs = len(out_avals)
    all_in_names = list(in_names) + list(out_names)
    if partition_name is not None:
        all_in_names.append(partition_name)

    def _body(*args):
        operands = list(args)
        if partition_name is not None:
            operands.append(partition_id_tensor())
        outs = _bass_exec_p.bind(
            *operands, out_avals=tuple(out_avals), in_names=tuple(all_in_names),
            out_names=tuple(out_names), lowering_input_output_aliases=(),
            sim_require_finite=True, sim_require_nnan=True, nc=nc)
        return tuple(outs)

    devices = jax.devices()[:n_cores]
    mesh = Mesh(np.asarray(devices), ("core",))
    in_specs = (PartitionSpec("core"),) * (n_params + n_outs)
    out_specs = (PartitionSpec("core"),) * n_outs
    jitted = jax.jit(shard_map(_body, mesh=mesh, in_specs=in_specs,
                               out_specs=out_specs, check_rep=False), keep_unused=True)

    def run(in_maps):
        concat_in = [np.concatenate([np.asarray(in_maps[c][n]) for c in range(n_cores)], axis=0)
                     for n in in_names]
        concat_zeros = [np.zeros((n_cores * z.shape[0], *z.shape[1:]), z.dtype) for z in zero_outs]
        outs = jitted(*concat_in, *concat_zeros)
        import jax as _jax
        _jax.block_until_ready(outs)
        return [{name: np.asarray(outs[i]).reshape(n_cores, *out_avals[i].shape)[c]
                 for i, name in enumerate(out_names)} for c in range(n_cores)]
    return run


def _get_runner(causal: bool):
    key = bool(causal)
    if key not in _CACHE:
        nc = _build_program(key)
        _CACHE[key] = _make_runner(nc, H)
    return _CACHE[key]


def _prep_head(q2, k2, v2, causal):
    """q2,k2,v2: [N, D] fp32 for one head. Returns per-core input dict."""
    import ml_dtypes
    qT = np.concatenate([q2.T, np.ones((1, N), np.float32)], axis=0)  # [65, N]
    kT = np.concatenate([k2.T, np.ones((1, N), np.float32)], axis=0)
    # v with ones col 64 (denominator) and zero col 65 (f32r even-width pad)
    v3 = np.concatenate([v2, np.ones((N, 1), np.float32),
                         np.zeros((N, 1), np.float32)], axis=1)      # [N, 66]
    v_r = np.ascontiguousarray(
        v3.reshape(NJC, 128, 66).transpose(1, 0, 2).reshape(128, NJC * 66)
    ).astype(ml_dtypes.bfloat16)
    if causal:
        vc = np.cumsum(v2, axis=0, dtype=np.float64).astype(np.float32)
    else:
        vc = np.broadcast_to(v2.sum(axis=0, dtype=np.float64).astype(np.float32), (N, 64))
    vcum = np.ascontiguousarray(
        vc.reshape(NT, 128, 64).transpose(1, 0, 2).reshape(128, NT * 64))
    return {"qT": np.ascontiguousarray(qT), "kT": np.ascontiguousarray(kT),
            "vr": v_r, "vcum": vcum}


def kernel(q, k, v, rpe_matrix, mask):
    causal = bool(np.asarray(mask).item()) if not isinstance(mask, (int, bool)) else bool(mask)
    q = np.asarray(q, dtype=np.float32)
    k = np.asarray(k, dtype=np.float32)
    v = np.asarray(v, dtype=np.float32)
    rpe = np.asarray(rpe_matrix, dtype=np.float32)

    RPW = 2560 if causal else 4608
    if causal:
        # u in [0, N-1]: rpe_rev[u] = rpe[2N-2-u] -> rows 2N-2 .. N-1
        rpe_rev = rpe[N - 1:2 * N - 1][::-1]             # [N, 64]
    else:
        rpe_rev = rpe[::-1]                              # [2N-1, 64]
    rpeT = np.zeros((65, RPW), dtype=np.float32)
    rpeT[0:64, :rpe_rev.shape[0]] = rpe_rev.T            # row 64 stays zero

    a = np.arange(128, dtype=np.float32)[:, None]
    tt = np.arange(NT, dtype=np.float32)[None, :]
    iota = (128 * tt + a + 1.0) if causal else np.full((128, NT), float(N), np.float32)
    iota = np.ascontiguousarray(iota.astype(np.float32))

    run = _get_runner(causal)
    in_maps = []
    for h in range(H):
        m = _prep_head(q[0, h], k[0, h], v[0, h], causal)
        m["rpeT"] = rpeT
        m["iota"] = iota
        in_maps.append(m)
    results = run(in_maps)
    # o stored [128, NT*64] with o_store[a, 64t+d] = o[128t+a, d]
    outs = []
    for h in range(H):
        oh = results[h]["o"].reshape(128, NT, 64).transpose(1, 0, 2).reshape(N, 64)
        outs.append(oh)
    out = np.stack(outs)[None]  # [1, H, N, 64]
    return out.astype(np.float32)


if __name__ == "__main__":
    rng = np.random.default_rng(0)
    q = rng.standard_normal((B, H, N, D), dtype=np.float32)
    k = rng.standard_normal((B, H, N, D), dtype=np.float32)
    v = rng.standard_normal((B, H, N, D), dtype=np.float32)
    rpe = rng.standard_normal((2 * N - 1, D), dtype=np.float32)
    o = kernel(q, k, v, rpe, 1)
    print("out", o.shape, o.dtype, np.abs(o).mean())
